# revision 6
# baseline (speedup 1.0000x reference)
"""Trainium2 Bass kernel for EnhancedOFTQKVLayer.

Computes out[b,s,o] = x[b,s,:] @ filt[o,:]^T + bias[o] where
filt = [Wq @ BD(cayley(q_R)); Wk @ BD(cayley(k_R)); Wv @ BD(cayley(v_R))]
(BD = block-diagonal, cayley(A) = (I-S) inv(I+S+eps I), S = 0.5(A-A^T)).

Distribution: data-parallel — batch b (8 rows) sharded one per NeuronCore;
attn_weight / bias / rotation blocks replicated. Per core:
  1. Cayley via SPD Newton-Schulz on P = (1+eps)^2 I - S^2 (iterates are
     polynomials in S^2 -> symmetric -> lhsT=operand works; periodic
     symmetrization kills fp16 roundoff drift). fp16 iters + fp32 polish.
  2. W^T built WITHOUT the tensor engine: W tiles stream in (gpsimd-dispatched
     DMA), are cast fp32->fp16 on gpsimd, then transposed by the DMA XBAR
     (dma_start(transpose=True)) into a resident [128, 8, 3072] W^T. filtT
     chunks = Q^T @ W^T on the PE, evicted to fp16.
  3. x tiles likewise: DMA -> gpsimd cast fp16 -> 8 XBAR transposes into
     [128, 8, 128] xT tiles. Phase C is then PURE 512-row fp16 matmuls
     (the bf16/fp16 PE roofline), og-outer so PSUM evictions (DVE, +bias)
     pipeline behind the matmul stream; out-DMAs split across the two
     HWDGE queues (sync / scalar).
"""

import numpy as np

import concourse.bass as bass
import concourse.mybir as mybir
import concourse.tile as tile
from concourse import bacc
from concourse.bass import ts
from concourse.masks import make_identity
from concourse.bass_utils import run_bass_kernel_spmd

F32 = mybir.dt.float32
F16 = mybir.dt.float16

MAIN_DT = F16            # dtype of the big matmul inputs (x, filtT)

HIDDEN = 1024
OUT_DIM = 3 * HIDDEN
SEQ = 4096
P = 128
NBLK = 8                 # 128-blocks per hidden
NROT = 24                # 3 * NBLK rotation blocks
EPS = 1e-6
N_CORES = 8

NSETS = 6                # Newton processes blocks in sets of 4
SETB = 4

# Newton-Schulz schedule (validated against the jax reference).
NEWTON_F16 = 8
NEWTON_F32 = 1
SYM_ITERS = {3, 5, 7}    # symmetrize on these fp16 iterations
X0_A = 0.0152174         # X0 = aI + bP (degree-1 minimax init on [1, 260])
X0_B = -5.78922e-05

M_TILES = SEQ // P       # 32
O_TILES = OUT_DIM // 512  # 6
XT_BUFS = 8              # in-flight transposed x tiles


def build_body(ctx, tc):
    nc = tc.nc

    x = nc.dram_tensor("x", [SEQ, HIDDEN], F32, kind="ExternalInput").ap()
    w = nc.dram_tensor("w", [OUT_DIM, HIDDEN], F32, kind="ExternalInput").ap()
    bias = nc.dram_tensor("bias", [OUT_DIM], F32, kind="ExternalInput").ap()
    rmat = nc.dram_tensor("rmat", [NROT, P, P], F32, kind="ExternalInput").ap()
    out = nc.dram_tensor("out", [SEQ, OUT_DIM], F32, kind="ExternalOutput").ap()

    sub = nc.vector.tensor_sub
    add = nc.vector.tensor_add
    smul = nc.vector.tensor_scalar_mul
    cp = nc.vector.tensor_copy
    acp = nc.any.tensor_copy

    def bc(t):  # broadcast a [P, P] constant over a set's middle dim
        return t[:].unsqueeze(1).to_broadcast([P, SETB, P])

    # ---- persistent pools ----
    const = ctx.enter_context(tc.tile_pool(name="const", bufs=1))
    ftp = ctx.enter_context(tc.tile_pool(name="ftp", bufs=1))
    xrp = ctx.enter_context(tc.tile_pool(name="xrp", bufs=2))
    xbp = ctx.enter_context(tc.tile_pool(name="xbp", bufs=2))
    xtp = ctx.enter_context(tc.tile_pool(name="xtp", bufs=XT_BUFS))
    obp = ctx.enter_context(tc.tile_pool(name="obp", bufs=3))

    ident32 = const.tile([P, P], F32)
    make_identity(nc, ident32)
    eI2 = const.tile([P, P], F32)       # (1+eps)^2 I
    smul(eI2[:], ident32[:], float((1.0 + EPS) ** 2))
    eI12 = const.tile([P, P], F32)      # ((1+eps) + (1+eps)^2) I
    smul(eI12[:], ident32[:], float((1.0 + EPS) + (1.0 + EPS) ** 2))
    twoI = const.tile([P, P], F32)      # 2 I
    smul(twoI[:], ident32[:], 2.0)
    aI0 = const.tile([P, P], F16)       # X0_A * I  (Newton init)
    smul(aI0[:], ident32[:], float(X0_A))
    two_eye16 = const.tile([P, P], F16)  # 2 I (fp16, Newton rhs)
    smul(two_eye16[:], ident32[:], 2.0)

    bias_bc = const.tile([P, OUT_DIM], MAIN_DT)
    with tc.tile_pool(name="biasld", bufs=1) as bl:
        brow = bl.tile([1, OUT_DIM], F32)
        nc.sync.dma_start(brow[:], bias.unsqueeze(0))
        cp(bias_bc[:1, :], brow[:])
    nc.gpsimd.partition_broadcast(bias_bc[:], bias_bc[:1, :])

    # filtT chunks: ft[k][og][c, o'] = filtT[k*128+c, og*512+o']
    ft = [[ftp.tile([P, 512], MAIN_DT, tag=f"ft{k}_{og}", name=f"ft{k}_{og}")
           for og in range(O_TILES)] for k in range(NBLK)]

    # x-tile prep: DMA -> gpsimd fp16 cast -> 8 XBAR transposes (no PE!)
    def emit_xprep(mt):
        xr = xrp.tile([P, HIDDEN], F32, tag="xr", name=f"xr{mt}")
        nc.sync.dma_start(xr[:], x[ts(mt, P), :])
        xb = xbp.tile([P, HIDDEN], MAIN_DT, tag="xb", name=f"xb{mt}")
        nc.gpsimd.tensor_copy(xb[:], xr[:])
        xt = xtp.tile([P, NBLK, P], MAIN_DT, tag="xt", name=f"xt{mt}")
        for k in range(NBLK):
            nc.sync.dma_start(xt[:, k, :], xb[:, ts(k, P)], transpose=True)
        return xt

    # ---- phase A+B scoped pools ----
    with (
        tc.tile_pool(name="nper", bufs=1) as nper,     # per-set persistents
        tc.tile_pool(name="nx", bufs=1) as nxp,        # per-set X iterates
        tc.tile_pool(name="nrot", bufs=2) as nrot,     # rotating temps
        tc.tile_pool(name="qpool", bufs=1) as qpool,
        tc.tile_pool(name="wstg", bufs=2) as wstg,
        tc.tile_pool(name="wbp", bufs=2) as wbp,
        tc.tile_pool(name="wtsp", bufs=3) as wtsp,
        tc.tile_pool(name="ps_g", bufs=6, space="PSUM") as ps_g,
        tc.tile_pool(name="ps_tp", bufs=2, space="PSUM") as ps_tp,
    ):
        # ---------- S-prep first so the rmat DMA heads the sync queue ----
        s_s, p32_s, p16_s, x_s = [], [], [], []
        aset_s, tpg_s = [], []
        for s in range(NSETS):
            n0 = s * SETB
            aset = nrot.tile([P, SETB, P], F32, tag="a")
            nc.sync.dma_start(aset[:],
                              rmat[n0:n0 + SETB].rearrange("n p f -> p n f"))
            tpg = ps_tp.tile([P, SETB, P], F32, tag="tp")
            for j in range(SETB):
                nc.tensor.transpose(tpg[:, j, :], aset[:, j, :], ident32[:])
            sset = nper.tile([P, SETB, P], F32, tag=f"s{s}", name=f"s{s}")
            sub(sset[:], aset[:], tpg[:])
            smul(sset[:], sset[:], 0.5)                  # S
            g = ps_g.tile([P, SETB, P], F32, tag="g")
            for j in range(SETB):                        # S^T @ S = -S^2
                nc.tensor.matmul(g[:, j, :], lhsT=sset[:, j, :],
                                 rhs=sset[:, j, :], start=True, stop=True)
            p32s = nper.tile([P, SETB, P], F32, tag=f"p32{s}", name=f"p32{s}")
            add(p32s[:], bc(eI2), g[:])                  # P = (1+e)^2 I - S^2
            p16s = nper.tile([P, SETB, P], F16, tag=f"p16{s}", name=f"p16{s}")
            acp(p16s[:], p32s[:])
            xset = nxp.tile([P, SETB, P], F16, tag=f"x{s}", name=f"x{s}_init")
            smul(xset[:], p32s[:], float(X0_B))          # X0 = aI + bP
            add(xset[:], xset[:], bc(aI0))
            s_s.append(sset)
            p32_s.append(p32s)
            p16_s.append(p16s)
            x_s.append(xset)

        # ---------- W path: DMA (gpsimd queue) -> cast -> XBAR ----------
        # W^T streamed in 512-col o-groups: wts[og][b, k, o'] (fp16)
        def emit_wprep(og):
            wts = wtsp.tile([P, NBLK, 512], MAIN_DT, tag="wts",
                            name=f"wts{og}")
            for j4 in range(4):             # four 128-row W tiles per group
                ot = og * 4 + j4
                wrow = wstg.tile([P, HIDDEN], F32, tag="wrow")
                nc.gpsimd.dma_start(wrow[:], w[ts(ot, P), :])
                wb = wbp.tile([P, HIDDEN], MAIN_DT, tag="wb")
                nc.gpsimd.tensor_copy(wb[:], wrow[:])
                for k in range(NBLK):
                    nc.scalar.dma_start(wts[:, k, ts(j4, P)], wb[:, ts(k, P)],
                                        transpose=True)
            return wts

        wts_pre = [emit_wprep(og) for og in range(3)]

        # prefetch x transposes (pure DMA/gpsimd work; overlaps Newton)
        xt_pre = [emit_xprep(mt) for mt in range(XT_BUFS)]

        # ---------- Newton-Schulz fp16 iterations ----------
        for i in range(NEWTON_F16):
            do_sym = i in SYM_ITERS
            for s in range(NSETS):
                g1 = ps_g.tile([P, SETB, P], F32, tag="g")
                for j in range(SETB):
                    nc.tensor.matmul(g1[:, j, :], lhsT=p16_s[s][:, j, :],
                                     rhs=x_s[s][:, j, :], start=True, stop=True)
                t1n = nrot.tile([P, SETB, P], F16, tag="t1n")
                nc.scalar.activation(t1n[:], g1[:],      # -T1, off the DVE
                                     mybir.ActivationFunctionType.Copy,
                                     scale=-1.0)
                g2 = ps_g.tile([P, SETB, P], F32, tag="g")
                for j in range(SETB):                    # X' = X(2I) - X T1
                    nc.tensor.matmul(g2[:, j, :], lhsT=x_s[s][:, j, :],
                                     rhs=two_eye16[:], start=True, stop=False)
                    nc.tensor.matmul(g2[:, j, :], lhsT=x_s[s][:, j, :],
                                     rhs=t1n[:, j, :], start=False, stop=True)
                xset = nxp.tile([P, SETB, P], F16, tag=f"x{s}",
                                name=f"x{s}_{i}")
                if not do_sym:
                    acp(xset[:], g2[:])
                else:
                    xc = nrot.tile([P, SETB, P], F32, tag="xc")
                    cp(xc[:], g2[:])
                    tpg = ps_tp.tile([P, SETB, P], F32, tag="tp")
                    for j in range(SETB):
                        nc.tensor.transpose(tpg[:, j, :], xc[:, j, :],
                                            ident32[:])
                    add(xc[:], xc[:], tpg[:])
                    nc.scalar.activation(xset[:], xc[:],
                                         mybir.ActivationFunctionType.Copy,
                                         scale=0.5)
                x_s[s] = xset

        xf_s = []
        for s in range(NSETS):
            xf = nxp.tile([P, SETB, P], F32, tag=f"xf{s}", name=f"xf{s}_init")
            acp(xf[:], x_s[s][:])
            xf_s.append(xf)
        for i in range(NEWTON_F32):
            for s in range(NSETS):
                g1 = ps_g.tile([P, SETB, P], F32, tag="g")
                for j in range(SETB):
                    nc.tensor.matmul(g1[:, j, :], lhsT=p32_s[s][:, j, :],
                                     rhs=xf_s[s][:, j, :], start=True,
                                     stop=True)
                uf = nrot.tile([P, SETB, P], F32, tag="uf")
                sub(uf[:], bc(twoI), g1[:])
                g2 = ps_g.tile([P, SETB, P], F32, tag="g")
                for j in range(SETB):
                    nc.tensor.matmul(g2[:, j, :], lhsT=xf_s[s][:, j, :],
                                     rhs=uf[:, j, :], start=True, stop=True)
                xf = nxp.tile([P, SETB, P], F32, tag=f"xf{s}",
                              name=f"xf{s}_{i}")
                acp(xf[:], g2[:])
                xf_s[s] = xf

        # Q = B @ X with B^T = (1+e)I + (2+e)S + S^2 = eI12 + (2+e)S - P
        q_s = []
        for s in range(NSETS):
            bt = nrot.tile([P, SETB, P], F32, tag="bt")
            nc.vector.tensor_scalar(bt[:], s_s[s][:], float(2.0 + EPS), None,
                                    mybir.AluOpType.mult)
            add(bt[:], bt[:], bc(eI12))
            sub(bt[:], bt[:], p32_s[s][:])
            g = ps_g.tile([P, SETB, P], F32, tag="g")
            for j in range(SETB):
                nc.tensor.matmul(g[:, j, :], lhsT=bt[:, j, :],
                                 rhs=xf_s[s][:, j, :], start=True, stop=True)
            qset = qpool.tile([P, SETB, P], MAIN_DT, tag=f"q{s}", name=f"q{s}")
            acp(qset[:], g[:])
            q_s.append(qset)

        def q_lhsT(n):
            return q_s[n // SETB][:, n % SETB, :]

        # ---------- Phase B: filtT = Q^T @ W^T over streamed W^T chunks ----
        for og in range(O_TILES):
            part = og // 2
            wts = wts_pre[og] if og < 3 else emit_wprep(og)
            for k in range(NBLK):
                fg = ps_g.tile([P, 512], F32, tag="g")
                nc.tensor.matmul(fg[:], lhsT=q_lhsT(part * NBLK + k),
                                 rhs=wts[:, k, :], start=True,
                                 stop=True)
                if k % 2 == 0:
                    nc.scalar.activation(ft[k][og][:], fg[:],
                                         mybir.ActivationFunctionType.Copy,
                                         scale=1.0)
                else:
                    cp(ft[k][og][:], fg[:])

        _CACHE["xt_pre"] = xt_pre

    # ---------- Phase C: pure matmul stream ----------
    xt_pre = _CACHE.pop("xt_pre")
    with tc.tile_pool(name="ps_out", bufs=8, space="PSUM") as ps_out:
        for mt in range(M_TILES):
            xt = xt_pre[mt] if mt < len(xt_pre) else emit_xprep(mt)
            if mt + XT_BUFS < M_TILES and mt + XT_BUFS >= len(xt_pre):
                xt_pre.append(emit_xprep(mt + XT_BUFS))
            for og in range(O_TILES):
                po = ps_out.tile([P, 512], F32, tag="po",
                                 name=f"po{mt}_{og}")
                for k in range(NBLK):
                    nc.tensor.matmul(po[:], lhsT=xt[:, k, :], rhs=ft[k][og][:],
                                     start=(k == 0), stop=(k == NBLK - 1))
                ob = obp.tile([P, 512], F32, tag="ob", name=f"ob{mt}_{og}")
                add(ob[:], po[:], bias_bc[:, ts(og, 512)])
                eng = nc.sync if og % 2 == 0 else nc.scalar
                eng.dma_start(out[ts(mt, P), ts(og, 512)], ob[:])


_CACHE = {}


def build():
    if "nc" in _CACHE:
        return _CACHE["nc"]
    import contextlib

    nc = bacc.Bacc("TRN2", target_bir_lowering=False, debug=False)
    with tile.TileContext(nc) as tc:
        with contextlib.ExitStack() as ctx:
            build_body(ctx, tc)
    nc.compile()
    _CACHE["nc"] = nc
    return nc


def make_in_maps(attn_weight, bias, x, q_R, k_R, v_R):
    rmat = np.ascontiguousarray(
        np.concatenate([q_R, k_R, v_R], axis=0), dtype=np.float32)
    w = np.ascontiguousarray(attn_weight, dtype=np.float32)
    b = np.ascontiguousarray(bias, dtype=np.float32)
    return [
        {"x": np.ascontiguousarray(x[c], dtype=np.float32),
         "w": w, "bias": b, "rmat": rmat}
        for c in range(N_CORES)
    ]


def kernel(attn_weight, bias, x, q_R, k_R, v_R, **run_kwargs):
    nc = build()
    in_maps = make_in_maps(attn_weight, bias, x, q_R, k_R, v_R)
    res = run_bass_kernel_spmd(nc, in_maps, core_ids=list(range(N_CORES)),
                               **run_kwargs)
    out = np.stack([res.results[c]["out"] for c in range(N_CORES)], axis=0)
    _CACHE["last_results"] = res
    return out


# revision 31
# speedup vs baseline: 2.2526x; 2.2526x over previous
"""Trainium2 Bass kernel for EnhancedOFTQKVLayer.

Computes out[b,s,o] = x[b,s,:] @ filt[o,:]^T + bias[o] where
filt = [Wq @ BD(cayley(q_R)); Wk @ BD(cayley(k_R)); Wv @ BD(cayley(v_R))]
(BD = block-diagonal, cayley(A) = (I-S) inv(I+S+eps I), S = 0.5(A-A^T)).

Distribution: data-parallel — batch b (8 rows) sharded one per NeuronCore;
attn_weight / bias / rotation blocks replicated. Per core:
  1. Cayley via SPD Newton-Schulz on P = (1+eps)^2 I - S^2. fp16 iterations
     write fp16 PSUM so the per-iteration elementwise ops run in the DVE/Act
     2x (16-bit) mode; the update uses T1' = 2I - P X on the DVE, halving the
     PE work per iteration vs. the X(2I) matmul trick. fp32 polish at the end.
  2. W^T tiles built by PE transposes (fp32 in -> fp16 PSUM, no separate cast)
     interleaved into the Newton iterations' spare PE slots; filtT chunks
     = Q^T @ W^T, evicted fp16.
  3. Main matmul fp16 (fp32 PSUM): x tiles DMA -> fp16 cast (scalar) ->
     PE transpose -> xT; phase C is a pure 512-row matmul stream at the
     16-bit PE roofline, og-outer so the PSUM evictions (DVE, +bias)
     trail each output group; out-DMAs split across both HWDGE queues.
"""

import numpy as np

import concourse.bass as bass
import concourse.mybir as mybir
import concourse.tile as tile
from concourse import bacc
from concourse.bass import ts
from concourse.masks import make_identity
from concourse.bass_utils import run_bass_kernel_spmd

F32 = mybir.dt.float32
F16 = mybir.dt.float16

MAIN_DT = F16            # dtype of the big matmul inputs (x, filtT)

HIDDEN = 1024
OUT_DIM = 3 * HIDDEN
SEQ = 4096
P = 128
NBLK = 8                 # 128-blocks per hidden
NROT = 24                # 3 * NBLK rotation blocks
EPS = 1e-6
N_CORES = 8

NSETS = 6                # Newton processes blocks in sets of 4
SETB = 4

# Newton-Schulz schedule (validated against the jax reference).
NEWTON_F16 = 8
NEWTON_F32 = 1
SYM_ITERS = {3, 5, 7}    # symmetrize on these fp16 iterations
X0_A = 0.0152174         # X0 = aI + bP (degree-1 minimax init on [1, 260])
X0_B = -5.78922e-05

M_TILES = SEQ // P       # 32
O_TILES = OUT_DIM // 512  # 6
XT_BUFS = 8              # in-flight transposed x tiles
PREFETCH = 8             # x tiles prepped during phase A
LOOKAHEAD = 2            # x tiles prepped ahead inside phase C


def build_body(ctx, tc):
    nc = tc.nc

    x = nc.dram_tensor("x", [SEQ, HIDDEN], F32, kind="ExternalInput").ap()
    w = nc.dram_tensor("w", [OUT_DIM, HIDDEN], F32, kind="ExternalInput").ap()
    bias = nc.dram_tensor("bias", [OUT_DIM], F32, kind="ExternalInput").ap()
    rmat = nc.dram_tensor("rmat", [NROT, P, P], F32, kind="ExternalInput").ap()
    out = nc.dram_tensor("out", [SEQ, OUT_DIM], F32, kind="ExternalOutput").ap()

    sub = nc.vector.tensor_sub
    add = nc.vector.tensor_add
    smul = nc.vector.tensor_scalar_mul
    cp = nc.vector.tensor_copy
    acp = nc.any.tensor_copy
    ACT_COPY = mybir.ActivationFunctionType.Copy

    def bc(t):  # broadcast a [P, P] constant over a set's middle dim
        return t[:].unsqueeze(1).to_broadcast([P, SETB, P])

    # ---- persistent pools ----
    const = ctx.enter_context(tc.tile_pool(name="const", bufs=1))
    ftp = ctx.enter_context(tc.tile_pool(name="ftp", bufs=1))
    xrp = ctx.enter_context(tc.tile_pool(name="xrp", bufs=2))
    xbp = ctx.enter_context(tc.tile_pool(name="xbp", bufs=2))
    xtp = ctx.enter_context(tc.tile_pool(name="xtp", bufs=XT_BUFS))
    obp = ctx.enter_context(tc.tile_pool(name="obp", bufs=3))
    ps_tp = ctx.enter_context(tc.tile_pool(name="ps_tp", bufs=2, space="PSUM"))

    ident32 = const.tile([P, P], F32)
    make_identity(nc, ident32)
    identb = const.tile([P, P], MAIN_DT)
    cp(identb[:], ident32[:])
    eI2 = const.tile([P, P], F32)       # (1+eps)^2 I
    smul(eI2[:], ident32[:], float((1.0 + EPS) ** 2))
    eI12 = const.tile([P, P], F32)      # ((1+eps) + (1+eps)^2) I
    smul(eI12[:], ident32[:], float((1.0 + EPS) + (1.0 + EPS) ** 2))
    twoI = const.tile([P, P], F32)      # 2 I
    smul(twoI[:], ident32[:], 2.0)
    aI0 = const.tile([P, P], F16)       # X0_A * I  (Newton init)
    smul(aI0[:], ident32[:], float(X0_A))
    two_eye16 = const.tile([P, P], F16)  # 2 I (fp16, Newton rhs)
    smul(two_eye16[:], ident32[:], 2.0)

    bias_bc = const.tile([P, OUT_DIM], MAIN_DT)
    with tc.tile_pool(name="biasld", bufs=1) as bl:
        brow = bl.tile([1, OUT_DIM], F32)
        nc.sync.dma_start(brow[:], bias.unsqueeze(0))
        cp(bias_bc[:1, :], brow[:])
    nc.gpsimd.partition_broadcast(bias_bc[:], bias_bc[:1, :])

    # filtT chunks: ft[k][og][c, o'] = filtT[k*128+c, og*512+o']
    ft = [[ftp.tile([P, 512], MAIN_DT, tag=f"ft{k}_{og}", name=f"ft{k}_{og}")
           for og in range(O_TILES)] for k in range(NBLK)]

    # x-tile prep: DMA -> fp16 cast (scalar) -> PE transpose -> DVE copy
    def emit_xprep(mt):
        xr = xrp.tile([P, HIDDEN], F32, tag="xr", name=f"xr{mt}")
        nc.sync.dma_start(xr[:], x[ts(mt, P), :])
        xb = xbp.tile([P, HIDDEN], MAIN_DT, tag="xb", name=f"xb{mt}")
        nc.scalar.activation(xb[:], xr[:], ACT_COPY, scale=1.0)
        tpg = ps_tp.tile([P, NBLK, P], MAIN_DT, tag="xtp", name=f"xtp{mt}")
        for k in range(NBLK):
            nc.tensor.transpose(tpg[:, k, :], xb[:, ts(k, P)], identb[:])
        xt = xtp.tile([P, NBLK, P], MAIN_DT, tag="xt", name=f"xt{mt}")
        cp(xt[:], tpg[:])
        return xt

    # ---- phase A+B scoped pools ----
    with (
        tc.tile_pool(name="nper", bufs=1) as nper,     # per-set persistents
        tc.tile_pool(name="nx", bufs=1) as nxp,        # per-set X iterates
        tc.tile_pool(name="nrot", bufs=2) as nrot,     # rotating temps
        tc.tile_pool(name="scr", bufs=1) as scr,       # polish/Q scratch
        tc.tile_pool(name="t1p", bufs=6) as t1p,       # Newton T1' ring
        tc.tile_pool(name="qpool", bufs=1) as qpool,
        tc.tile_pool(name="wstg", bufs=2) as wstg,
        tc.tile_pool(name="wtsp", bufs=2) as wtsp,
        tc.tile_pool(name="ps_g", bufs=4, space="PSUM") as ps_g,
        tc.tile_pool(name="ps_f", bufs=2, space="PSUM") as ps_f,
    ):
        # ---------- S-prep (stage-wise in waves of 3 sets) ----------
        # Work with D = A - A^T (S = D/2); scale factors folded into the
        # fused scalar_tensor_tensor ops: P = (1+e)^2 I + 0.25 D^T D,
        # X0 = aI + bP, c32 = eI12 - P (Q-time helper, on gpsimd).
        stt = nc.vector.scalar_tensor_tensor
        MUL, ADD = mybir.AluOpType.mult, mybir.AluOpType.add
        d_s, p32_s, p16_s, x_s, c32_s = [], [], [], [], []
        aset_s, tpg_s, g_s = {}, {}, {}
        for s in range(NSETS):
            n0 = s * SETB
            aset = nrot.tile([P, SETB, P], F32, tag=f"a{s % 3}")
            nc.sync.dma_start(aset[:],
                              rmat[n0:n0 + SETB].rearrange("n p f -> p n f"))
            aset_s[s] = aset
        for w0 in range(0, NSETS, 3):
            wave = range(w0, w0 + 3)
            for s in wave:
                tpg = ps_f.tile([P, SETB, P], F32, tag="gf")
                for j in range(SETB):
                    nc.tensor.transpose(tpg[:, j, :], aset_s[s][:, j, :],
                                        ident32[:])
                tpg_s[s] = tpg
            for s in wave:
                dset = nper.tile([P, SETB, P], F32, tag=f"s{s}", name=f"d{s}")
                sub(dset[:], aset_s[s][:], tpg_s[s][:])  # D = A - A^T
                d_s.append(dset)
            for s in wave:
                g = ps_f.tile([P, SETB, P], F32, tag="gf")
                for j in range(SETB):                    # D^T D = -D^2
                    nc.tensor.matmul(g[:, j, :], lhsT=d_s[s][:, j, :],
                                     rhs=d_s[s][:, j, :], start=True,
                                     stop=True)
                g_s[s] = g
            for s in wave:
                p32s = nper.tile([P, SETB, P], F32, tag=f"p32{s}",
                                 name=f"p32{s}")
                stt(p32s[:], g_s[s][:], 0.25, bc(eI2), MUL, ADD)
                p32_s.append(p32s)
            for s in wave:
                p16s = nper.tile([P, SETB, P], F16, tag=f"p16{s}",
                                 name=f"p16{s}")
                nc.scalar.activation(p16s[:], p32_s[s][:], ACT_COPY, scale=1.0)
                p16_s.append(p16s)
            for s in wave:
                xset = nxp.tile([P, SETB, P], F16, tag=f"x{s}",
                                name=f"x{s}_init")
                stt(xset[:], p32_s[s][:], float(X0_B), bc(aI0), MUL, ADD)
                x_s.append(xset)
            for s in wave:   # c32 = eI12 - P (off the Newton critical path)
                c32 = aset_s[s]  # reuses the dead aset buffer
                stt(c32[:], p32_s[s][:], -1.0, bc(eI12), MUL, ADD)
                c32_s.append(c32)

        # W^T chunk builder: PE transpose (fp32 in -> fp16 PSUM), evictions
        # alternate scalar/DVE. wts[og][b, k, o'] fp16.
        wrows = {}

        def emit_wload(ot):
            wrow = wstg.tile([P, HIDDEN], F32, tag="wrow", name=f"w{ot}")
            nc.gpsimd.dma_start(wrow[:], w[ts(ot, P), :])
            wrows[ot] = wrow

        def emit_wtile(wts, ot):
            """Transpose one 128-row W tile into wts[:, :, ts(ot%4, P)]."""
            j4 = ot % 4
            wrow = wrows.pop(ot)
            for kh in range(2):
                tpg = ps_f.tile([P, SETB, P], F32, tag="gf")
                for k4 in range(SETB):
                    k = kh * SETB + k4
                    nc.tensor.transpose(tpg[:, k4, :], wrow[:, ts(k, P)],
                                        ident32[:])
                dst = wts[:, ts(kh, SETB), ts(j4, P)]
                if (ot + kh) % 2 == 0:
                    nc.scalar.activation(dst, tpg[:], ACT_COPY, scale=1.0)
                else:
                    cp(dst, tpg[:])

        wts_all = {og: wtsp.tile([P, NBLK, 512], MAIN_DT, tag="wts",
                                 name=f"wts{og}") for og in range(2)}

        # prefetch x tiles (DMA + scalar cast + PE transpose; fills prologue)
        xt_pre = [emit_xprep(mt) for mt in range(PREFETCH)]

        for ot in range(4):
            emit_wload(ot)

        # ---------- Newton-Schulz fp16 iterations (stage-wise) ----------
        # per round: all g1 = P X; then per set either
        #   DVE path  (s odd):  T1' = 2I - g1 (DVE); g2 = X T1'   (8 mm)
        #   Act path  (s even): T1n = -g1 (scalar); g2 = X 2I + X T1n (12 mm)
        # so the PSUM-read elementwise load splits across both engines.
        for i in range(NEWTON_F16):
            do_sym = i in SYM_ITERS
            g1_s, t1_s, g2_s = {}, {}, {}
            for s in range(NSETS):
                g1 = ps_g.tile([P, SETB, P], F32, tag="g")
                for j in range(SETB):
                    nc.tensor.matmul(g1[:, j, :], lhsT=p16_s[s][:, j, :],
                                     rhs=x_s[s][:, j, :], start=True, stop=True)
                g1_s[s] = g1
            for s in range(NSETS):
                t1n = t1p.tile([P, SETB, P], F16, tag="t1n")
                if s % 2 == 0:
                    nc.scalar.activation(t1n[:], g1_s[s][:], ACT_COPY,
                                         scale=-1.0)     # -P X
                else:
                    sub(t1n[:], bc(two_eye16), g1_s[s][:])  # 2I - P X
                t1_s[s] = t1n
            for s in range(NSETS):
                g2 = ps_g.tile([P, SETB, P], F32, tag="g")
                if s % 2 == 0:
                    for j in range(SETB):
                        nc.tensor.matmul(g2[:, j, :], lhsT=x_s[s][:, j, :],
                                         rhs=two_eye16[:], start=True,
                                         stop=False)
                        nc.tensor.matmul(g2[:, j, :], lhsT=x_s[s][:, j, :],
                                         rhs=t1_s[s][:, j, :], start=False,
                                         stop=True)
                else:
                    for j in range(SETB):
                        nc.tensor.matmul(g2[:, j, :], lhsT=x_s[s][:, j, :],
                                         rhs=t1_s[s][:, j, :], start=True,
                                         stop=True)
                g2_s[s] = g2
            scale = 0.5 if do_sym else 1.0
            xnew = {}
            for s in range(NSETS):
                xset = nxp.tile([P, SETB, P], F16, tag=f"x{s}",
                                name=f"x{s}_{i}")
                if s % 2 == 0:
                    nc.scalar.activation(xset[:], g2_s[s][:], ACT_COPY,
                                         scale=scale)
                elif do_sym:
                    nc.vector.tensor_scalar(xset[:], g2_s[s][:], 0.5, None,
                                            mybir.AluOpType.mult)
                else:
                    cp(xset[:], g2_s[s][:])
                xnew[s] = xset
            if do_sym:
                tp_s = {}
                for s in range(NSETS):
                    tpg = ps_tp.tile([P, NBLK, P], F16, tag="xtp")
                    for j in range(SETB):
                        nc.tensor.transpose(tpg[:, j, :], xnew[s][:, j, :],
                                            identb[:])
                    tp_s[s] = tpg
                for s in range(NSETS):
                    xsym = nxp.tile([P, SETB, P], F16, tag=f"xs{s}",
                                    name=f"x{s}_{i}s")
                    add(xsym[:], xnew[s][:], tp_s[s][:, :SETB, :])
                    xnew[s] = xsym
            for s in range(NSETS):
                x_s[s] = xnew[s]
            # fill spare PE slots with W transposes for og 0..1
            if i < 4:
                for ot in range(2 * i, 2 * i + 2):
                    emit_wtile(wts_all[ot // 4], ot)
                    if ot + 4 < 8:
                        emit_wload(ot + 4)

        # ---------- fp32 polish (stage-wise in waves of 3) ----------
        xf_s = []
        for s in range(NSETS):
            xf = nxp.tile([P, SETB, P], F32, tag=f"xf{s}", name=f"xf{s}_init")
            if s % 2 == 0:
                nc.scalar.activation(xf[:], x_s[s][:], ACT_COPY, scale=1.0)
            else:
                cp(xf[:], x_s[s][:])
            xf_s.append(xf)
        for i in range(NEWTON_F32):
            for w0 in range(0, NSETS, 3):
                wave = range(w0, w0 + 3)
                g1_s, uf_s, g2_s = {}, {}, {}
                for s in wave:
                    g1 = ps_f.tile([P, SETB, P], F32, tag="gf")
                    for j in range(SETB):
                        nc.tensor.matmul(g1[:, j, :], lhsT=p32_s[s][:, j, :],
                                         rhs=xf_s[s][:, j, :], start=True,
                                         stop=True)
                    g1_s[s] = g1
                for s in wave:
                    uf = scr.tile([P, SETB, P], F32, tag=f"sc{s % 3}")
                    sub(uf[:], bc(twoI), g1_s[s][:])
                    uf_s[s] = uf
                for s in wave:
                    g2 = ps_f.tile([P, SETB, P], F32, tag="gf")
                    for j in range(SETB):
                        nc.tensor.matmul(g2[:, j, :], lhsT=xf_s[s][:, j, :],
                                         rhs=uf_s[s][:, j, :], start=True,
                                         stop=True)
                    g2_s[s] = g2
                for s in wave:
                    xf = nxp.tile([P, SETB, P], F32, tag=f"xf{s}",
                                  name=f"xf{s}_{i}")
                    if s % 2 == 0:
                        nc.scalar.activation(xf[:], g2_s[s][:], ACT_COPY,
                                             scale=1.0)
                    else:
                        cp(xf[:], g2_s[s][:])
                    xf_s[s] = xf

        # Q = B @ X with B^T = (2+e)S + (eI12 - P) = 0.5(2+e) D + c32
        q_s = []
        for w0 in range(0, NSETS, 3):
            wave = range(w0, w0 + 3)
            bt_s, gq_s = {}, {}
            for s in wave:
                bt = scr.tile([P, SETB, P], F32, tag=f"sc{s % 3}")
                stt(bt[:], d_s[s][:], float(0.5 * (2.0 + EPS)), c32_s[s][:],
                    MUL, ADD)
                bt_s[s] = bt
            for s in wave:
                g = ps_f.tile([P, SETB, P], F32, tag="gf")
                for j in range(SETB):
                    nc.tensor.matmul(g[:, j, :], lhsT=bt_s[s][:, j, :],
                                     rhs=xf_s[s][:, j, :], start=True,
                                     stop=True)
                gq_s[s] = g
            for s in wave:
                qset = qpool.tile([P, SETB, P], MAIN_DT, tag=f"q{s}",
                                  name=f"q{s}")
                if s % 2 == 0:
                    nc.scalar.activation(qset[:], gq_s[s][:], ACT_COPY,
                                         scale=1.0)
                else:
                    cp(qset[:], gq_s[s][:])
                q_s.append(qset)

        def q_lhsT(n):
            return q_s[n // SETB][:, n % SETB, :]

        # ---------- Phase B: filtT = Q^T @ W^T over streamed W^T chunks ----
        for og in range(O_TILES):
            part = og // 2
            if og >= 2:
                wts = wtsp.tile([P, NBLK, 512], MAIN_DT, tag="wts",
                                name=f"wts{og}")
                for j4 in range(4):
                    ot = og * 4 + j4
                    emit_wload(ot)
                    emit_wtile(wts, ot)
            else:
                wts = wts_all[og]
            for k in range(NBLK):
                fg = ps_f.tile([P, 512], F32, tag="gf")
                nc.tensor.matmul(fg[:], lhsT=q_lhsT(part * NBLK + k),
                                 rhs=wts[:, k, :], start=True, stop=True)
                if k % 2 == 0:
                    nc.scalar.activation(ft[k][og][:], fg[:], ACT_COPY,
                                         scale=1.0)
                else:
                    cp(ft[k][og][:], fg[:])

        _CACHE["xt_pre"] = xt_pre

    # ---------- Phase C: matmul stream at the 16-bit PE roofline ----------
    xt_pre = _CACHE.pop("xt_pre")
    with tc.tile_pool(name="ps_out", bufs=6, space="PSUM") as ps_out:
        for mt in range(M_TILES):
            la = mt + LOOKAHEAD
            if PREFETCH <= la < M_TILES:
                xt_pre.append(emit_xprep(la))
            xt = xt_pre[mt]
            for og in range(O_TILES):
                po = ps_out.tile([P, 512], F32, tag="po",
                                 name=f"po{mt}_{og}")
                for k in range(NBLK):
                    nc.tensor.matmul(po[:], lhsT=xt[:, k, :], rhs=ft[k][og][:],
                                     start=(k == 0), stop=(k == NBLK - 1))
                ob = obp.tile([P, 512], F32, tag="ob", name=f"ob{mt}_{og}")
                add(ob[:], po[:], bias_bc[:, ts(og, 512)])
                eng = nc.sync if og % 2 == 0 else nc.scalar
                eng.dma_start(out[ts(mt, P), ts(og, 512)], ob[:])


_CACHE = {}


def build():
    if "nc" in _CACHE:
        return _CACHE["nc"]
    import contextlib

    nc = bacc.Bacc("TRN2", target_bir_lowering=False, debug=False)
    with tile.TileContext(nc) as tc:
        with contextlib.ExitStack() as ctx:
            build_body(ctx, tc)
    nc.compile()
    _CACHE["nc"] = nc
    return nc


def make_in_maps(attn_weight, bias, x, q_R, k_R, v_R):
    rmat = np.ascontiguousarray(
        np.concatenate([q_R, k_R, v_R], axis=0), dtype=np.float32)
    w = np.ascontiguousarray(attn_weight, dtype=np.float32)
    b = np.ascontiguousarray(bias, dtype=np.float32)
    return [
        {"x": np.ascontiguousarray(x[c], dtype=np.float32),
         "w": w, "bias": b, "rmat": rmat}
        for c in range(N_CORES)
    ]


def kernel(attn_weight, bias, x, q_R, k_R, v_R, **run_kwargs):
    nc = build()
    in_maps = make_in_maps(attn_weight, bias, x, q_R, k_R, v_R)
    res = run_bass_kernel_spmd(nc, in_maps, core_ids=list(range(N_CORES)),
                               **run_kwargs)
    out = np.stack([res.results[c]["out"] for c in range(N_CORES)], axis=0)
    _CACHE["last_results"] = res
    return out


# revision 38
# speedup vs baseline: 2.3585x; 1.0470x over previous
"""Trainium2 Bass kernel for EnhancedOFTQKVLayer.

Computes out[b,s,o] = x[b,s,:] @ filt[o,:]^T + bias[o] where
filt = [Wq @ BD(cayley(q_R)); Wk @ BD(cayley(k_R)); Wv @ BD(cayley(v_R))]
(BD = block-diagonal, cayley(A) = (I-S) inv(I+S+eps I), S = 0.5(A-A^T)).

Distribution: data-parallel — batch b (8 rows) sharded one per NeuronCore;
attn_weight / bias / rotation blocks replicated. Per core:
  1. Cayley via SPD Newton-Schulz on P = (1+eps)^2 I - S^2. fp16 iterations
     write fp16 PSUM so the per-iteration elementwise ops run in the DVE/Act
     2x (16-bit) mode; the update uses T1' = 2I - P X on the DVE, halving the
     PE work per iteration vs. the X(2I) matmul trick. fp32 polish at the end.
  2. W^T tiles built by PE transposes (fp32 in -> fp16 PSUM, no separate cast)
     interleaved into the Newton iterations' spare PE slots; filtT chunks
     = Q^T @ W^T, evicted fp16.
  3. Main matmul fp16 (fp32 PSUM): x tiles DMA -> fp16 cast (scalar) ->
     PE transpose -> xT; phase C is a pure 512-row matmul stream at the
     16-bit PE roofline, og-outer so the PSUM evictions (DVE, +bias)
     trail each output group; out-DMAs split across both HWDGE queues.
"""

import numpy as np

import concourse.bass as bass
import concourse.mybir as mybir
import concourse.tile as tile
from concourse import bacc
from concourse.bass import ts
from concourse.masks import make_identity
from concourse.bass_utils import run_bass_kernel_spmd

F32 = mybir.dt.float32
F16 = mybir.dt.float16

MAIN_DT = F16            # dtype of the big matmul inputs (x, filtT)

HIDDEN = 1024
OUT_DIM = 3 * HIDDEN
SEQ = 4096
P = 128
NBLK = 8                 # 128-blocks per hidden
NROT = 24                # 3 * NBLK rotation blocks
EPS = 1e-6
N_CORES = 8

NSETS = 6                # Newton processes blocks in sets of 4
SETB = 4

# Newton-Schulz schedule (validated against the jax reference).
NEWTON_F16 = 8
NEWTON_F32 = 1
SYM_ITERS = {3, 5, 7}    # symmetrize on these fp16 iterations
X0_A = 0.0152174         # X0 = aI + bP (degree-1 minimax init on [1, 260])
X0_B = -5.78922e-05

M_TILES = SEQ // P       # 32
O_TILES = OUT_DIM // 512  # 6
XT_BUFS = 10             # in-flight transposed x tiles
PREFETCH = 8             # x tiles prepped during phase A
LOOKAHEAD = 2            # x tiles prepped ahead inside phase C


def build_body(ctx, tc):
    nc = tc.nc

    x = nc.dram_tensor("x", [SEQ, HIDDEN], F32, kind="ExternalInput").ap()
    w = nc.dram_tensor("w", [OUT_DIM, HIDDEN], F32, kind="ExternalInput").ap()
    bias = nc.dram_tensor("bias", [OUT_DIM], F32, kind="ExternalInput").ap()
    rmat = nc.dram_tensor("rmat", [NROT, P, P], F32, kind="ExternalInput").ap()
    out = nc.dram_tensor("out", [SEQ, OUT_DIM], F32, kind="ExternalOutput").ap()

    sub = nc.vector.tensor_sub
    add = nc.vector.tensor_add
    smul = nc.vector.tensor_scalar_mul
    cp = nc.vector.tensor_copy
    acp = nc.any.tensor_copy
    ACT_COPY = mybir.ActivationFunctionType.Copy

    def bc(t):  # broadcast a [P, P] constant over a set's middle dim
        return t[:].unsqueeze(1).to_broadcast([P, SETB, P])

    # ---- persistent pools ----
    const = ctx.enter_context(tc.tile_pool(name="const", bufs=1))
    ftp = ctx.enter_context(tc.tile_pool(name="ftp", bufs=1))
    xrp = ctx.enter_context(tc.tile_pool(name="xrp", bufs=2))
    xbp = ctx.enter_context(tc.tile_pool(name="xbp", bufs=2))
    xtp = ctx.enter_context(tc.tile_pool(name="xtp", bufs=XT_BUFS))
    obp = ctx.enter_context(tc.tile_pool(name="obp", bufs=3))
    ps_tp = ctx.enter_context(tc.tile_pool(name="ps_tp", bufs=2, space="PSUM"))

    ident32 = const.tile([P, P], F32)
    make_identity(nc, ident32)
    identb = const.tile([P, P], MAIN_DT)
    cp(identb[:], ident32[:])
    eI2 = const.tile([P, P], F32)       # (1+eps)^2 I
    smul(eI2[:], ident32[:], float((1.0 + EPS) ** 2))
    eI12 = const.tile([P, P], F32)      # ((1+eps) + (1+eps)^2) I
    smul(eI12[:], ident32[:], float((1.0 + EPS) + (1.0 + EPS) ** 2))
    twoI = const.tile([P, P], F32)      # 2 I
    smul(twoI[:], ident32[:], 2.0)
    aI0 = const.tile([P, P], F16)       # X0_A * I  (Newton init)
    smul(aI0[:], ident32[:], float(X0_A))
    two_eye16 = const.tile([P, P], F16)  # 2 I (fp16, Newton rhs)
    smul(two_eye16[:], ident32[:], 2.0)

    bias_bc = const.tile([P, OUT_DIM], MAIN_DT)
    with tc.tile_pool(name="biasld", bufs=1) as bl:
        brow = bl.tile([1, OUT_DIM], F32)
        nc.sync.dma_start(brow[:], bias.unsqueeze(0))
        cp(bias_bc[:1, :], brow[:])
    nc.gpsimd.partition_broadcast(bias_bc[:], bias_bc[:1, :])

    # filtT chunks: FT[og][c, k, o'] = filtT[k*128+c, og*512+o'].
    # First filled with W^T (streamed during Newton), then overwritten
    # in place by Q^T @ W^T in phase B.
    FT = [ftp.tile([P, NBLK, 512], MAIN_DT, tag=f"ft{og}", name=f"ft{og}")
          for og in range(O_TILES)]

    # x-tile prep: DMA -> fp16 cast (scalar) -> PE transpose -> DVE copy
    def emit_xprep(mt):
        xr = xrp.tile([P, HIDDEN], F32, tag="xr", name=f"xr{mt}")
        nc.sync.dma_start(xr[:], x[ts(mt, P), :])
        xb = xbp.tile([P, HIDDEN], MAIN_DT, tag="xb", name=f"xb{mt}")
        nc.scalar.activation(xb[:], xr[:], ACT_COPY, scale=1.0)
        tpg = ps_tp.tile([P, NBLK, P], MAIN_DT, tag="xtp", name=f"xtp{mt}")
        for k in range(NBLK):
            nc.tensor.transpose(tpg[:, k, :], xb[:, ts(k, P)], identb[:])
        xt = xtp.tile([P, NBLK, P], MAIN_DT, tag="xt", name=f"xt{mt}")
        cp(xt[:], tpg[:])
        return xt

    # ---- phase A+B scoped pools ----
    with (
        tc.tile_pool(name="nper", bufs=1) as nper,     # per-set persistents
        tc.tile_pool(name="nx", bufs=1) as nxp,        # per-set X iterates
        tc.tile_pool(name="nrot", bufs=2) as nrot,     # rotating temps
        tc.tile_pool(name="scr", bufs=1) as scr,       # polish/Q scratch
        tc.tile_pool(name="t1p", bufs=6) as t1p,       # Newton T1' ring
        tc.tile_pool(name="qpool", bufs=1) as qpool,
        tc.tile_pool(name="wstg", bufs=3) as wstg,
        tc.tile_pool(name="ps_g", bufs=4, space="PSUM") as ps_g,
        tc.tile_pool(name="ps_f", bufs=2, space="PSUM") as ps_f,
    ):
        # ---------- S-prep (stage-wise in waves of 3 sets) ----------
        # Work with D = A - A^T (S = D/2); scale factors folded into the
        # fused scalar_tensor_tensor ops: P = (1+e)^2 I + 0.25 D^T D,
        # X0 = aI + bP, c32 = eI12 - P (Q-time helper, on gpsimd).
        stt = nc.vector.scalar_tensor_tensor
        MUL, ADD = mybir.AluOpType.mult, mybir.AluOpType.add
        d_s, p32_s, p16_s, x_s, c32_s = [], [], [], [], []
        aset_s, tpg_s, g_s = {}, {}, {}
        for s in range(NSETS):
            n0 = s * SETB
            aset = nrot.tile([P, SETB, P], F32, tag=f"a{s % 3}")
            nc.sync.dma_start(aset[:],
                              rmat[n0:n0 + SETB].rearrange("n p f -> p n f"))
            aset_s[s] = aset
        for w0 in range(0, NSETS, 3):
            wave = range(w0, w0 + 3)
            for s in wave:
                tpg = ps_f.tile([P, SETB, P], F32, tag="gf")
                for j in range(SETB):
                    nc.tensor.transpose(tpg[:, j, :], aset_s[s][:, j, :],
                                        ident32[:])
                tpg_s[s] = tpg
            for s in wave:
                dset = nper.tile([P, SETB, P], F32, tag=f"s{s}", name=f"d{s}")
                sub(dset[:], aset_s[s][:], tpg_s[s][:])  # D = A - A^T
                d_s.append(dset)
            for s in wave:
                g = ps_f.tile([P, SETB, P], F32, tag="gf")
                for j in range(SETB):                    # D^T D = -D^2
                    nc.tensor.matmul(g[:, j, :], lhsT=d_s[s][:, j, :],
                                     rhs=d_s[s][:, j, :], start=True,
                                     stop=True)
                g_s[s] = g
            for s in wave:
                p32s = nper.tile([P, SETB, P], F32, tag=f"p32{s}",
                                 name=f"p32{s}")
                stt(p32s[:], g_s[s][:], 0.25, bc(eI2), MUL, ADD)
                p32_s.append(p32s)
            for s in wave:
                p16s = nper.tile([P, SETB, P], F16, tag=f"p16{s}",
                                 name=f"p16{s}")
                nc.scalar.activation(p16s[:], p32_s[s][:], ACT_COPY, scale=1.0)
                p16_s.append(p16s)
            for s in wave:
                xset = nxp.tile([P, SETB, P], F16, tag=f"x{s}",
                                name=f"x{s}_init")
                stt(xset[:], p32_s[s][:], float(X0_B), bc(aI0), MUL, ADD)
                x_s.append(xset)
            for s in wave:   # c32 = eI12 - P (off the Newton critical path)
                c32 = aset_s[s]  # reuses the dead aset buffer
                stt(c32[:], p32_s[s][:], -1.0, bc(eI12), MUL, ADD)
                c32_s.append(c32)

        # W^T builder: PE transpose (fp32) -> evict fp16 straight into the
        # FT[og] tile that phase B later overwrites in place with Q^T W^T.
        wrows = {}

        def emit_wload(ot):
            wrow = wstg.tile([P, HIDDEN], F32, tag="wrow", name=f"w{ot}")
            nc.gpsimd.dma_start(wrow[:], w[ts(ot, P), :])
            wrows[ot] = wrow

        def emit_wtile(ot):
            """Transpose one 128-row W tile into FT[ot//4][:, :, ts(ot%4, P)]."""
            og, j4 = ot // 4, ot % 4
            wrow = wrows.pop(ot)
            for kh in range(2):
                tpg = ps_f.tile([P, SETB, P], F32, tag="gf")
                for k4 in range(SETB):
                    k = kh * SETB + k4
                    nc.tensor.transpose(tpg[:, k4, :], wrow[:, ts(k, P)],
                                        ident32[:])
                dst = FT[og][:, ts(kh, SETB), ts(j4, P)]
                if (ot + kh) % 2 == 0:
                    nc.scalar.activation(dst, tpg[:], ACT_COPY, scale=1.0)
                else:
                    cp(dst, tpg[:])

        # prefetch x tiles (DMA + scalar cast + PE transpose; fills prologue)
        xt_pre = [emit_xprep(mt) for mt in range(PREFETCH)]

        for ot in range(3):
            emit_wload(ot)

        # ---------- Newton-Schulz fp16 iterations (stage-wise) ----------
        # per round: all g1 = P X; then per set either
        #   DVE path  (s odd):  T1' = 2I - g1 (DVE); g2 = X T1'   (8 mm)
        #   Act path  (s even): T1n = -g1 (scalar); g2 = X 2I + X T1n (12 mm)
        # so the PSUM-read elementwise load splits across both engines.
        for i in range(NEWTON_F16):
            do_sym = i in SYM_ITERS
            g1_s, t1_s, g2_s = {}, {}, {}
            for s in range(NSETS):
                g1 = ps_g.tile([P, SETB, P], F32, tag="g")
                for j in range(SETB):
                    nc.tensor.matmul(g1[:, j, :], lhsT=p16_s[s][:, j, :],
                                     rhs=x_s[s][:, j, :], start=True, stop=True)
                g1_s[s] = g1
            for s in range(NSETS):
                t1n = t1p.tile([P, SETB, P], F16, tag="t1n")
                if s % 2 == 0:
                    nc.scalar.activation(t1n[:], g1_s[s][:], ACT_COPY,
                                         scale=-1.0)     # -P X
                else:
                    sub(t1n[:], bc(two_eye16), g1_s[s][:])  # 2I - P X
                t1_s[s] = t1n
            for s in range(NSETS):
                g2 = ps_g.tile([P, SETB, P], F32, tag="g")
                if s % 2 == 0:
                    for j in range(SETB):
                        nc.tensor.matmul(g2[:, j, :], lhsT=x_s[s][:, j, :],
                                         rhs=two_eye16[:], start=True,
                                         stop=False)
                        nc.tensor.matmul(g2[:, j, :], lhsT=x_s[s][:, j, :],
                                         rhs=t1_s[s][:, j, :], start=False,
                                         stop=True)
                else:
                    for j in range(SETB):
                        nc.tensor.matmul(g2[:, j, :], lhsT=x_s[s][:, j, :],
                                         rhs=t1_s[s][:, j, :], start=True,
                                         stop=True)
                g2_s[s] = g2
            scale = 0.5 if do_sym else 1.0
            xnew = {}
            for s in range(NSETS):
                xset = nxp.tile([P, SETB, P], F16, tag=f"x{s}",
                                name=f"x{s}_{i}")
                if s % 2 == 0:
                    nc.scalar.activation(xset[:], g2_s[s][:], ACT_COPY,
                                         scale=scale)
                elif do_sym:
                    nc.vector.tensor_scalar(xset[:], g2_s[s][:], 0.5, None,
                                            mybir.AluOpType.mult)
                else:
                    cp(xset[:], g2_s[s][:])
                xnew[s] = xset
            if do_sym:
                tp_s = {}
                for s in range(NSETS):
                    tpg = ps_tp.tile([P, NBLK, P], F16, tag="xtp")
                    for j in range(SETB):
                        nc.tensor.transpose(tpg[:, j, :], xnew[s][:, j, :],
                                            identb[:])
                    tp_s[s] = tpg
                for s in range(NSETS):
                    xsym = nxp.tile([P, SETB, P], F16, tag=f"xs{s}",
                                    name=f"x{s}_{i}s")
                    add(xsym[:], xnew[s][:], tp_s[s][:, :SETB, :])
                    xnew[s] = xsym
            for s in range(NSETS):
                x_s[s] = xnew[s]
            # fill spare PE slots with W transposes (3 row-tiles per round)
            for ot in range(3 * i, 3 * i + 3):
                emit_wtile(ot)
                if ot + 3 < OUT_DIM // P:
                    emit_wload(ot + 3)

        # ---------- fp32 polish (stage-wise in waves of 3) ----------
        xf_s = []
        for s in range(NSETS):
            xf = nxp.tile([P, SETB, P], F32, tag=f"xf{s}", name=f"xf{s}_init")
            if s % 2 == 0:
                nc.scalar.activation(xf[:], x_s[s][:], ACT_COPY, scale=1.0)
            else:
                cp(xf[:], x_s[s][:])
            xf_s.append(xf)
        for i in range(NEWTON_F32):
            for w0 in range(0, NSETS, 3):
                wave = range(w0, w0 + 3)
                g1_s, uf_s, g2_s = {}, {}, {}
                for s in wave:
                    g1 = ps_f.tile([P, SETB, P], F32, tag="gf")
                    for j in range(SETB):
                        nc.tensor.matmul(g1[:, j, :], lhsT=p32_s[s][:, j, :],
                                         rhs=xf_s[s][:, j, :], start=True,
                                         stop=True)
                    g1_s[s] = g1
                for s in wave:
                    uf = scr.tile([P, SETB, P], F32, tag=f"sc{s % 3}")
                    sub(uf[:], bc(twoI), g1_s[s][:])
                    uf_s[s] = uf
                for s in wave:
                    g2 = ps_f.tile([P, SETB, P], F32, tag="gf")
                    for j in range(SETB):
                        nc.tensor.matmul(g2[:, j, :], lhsT=xf_s[s][:, j, :],
                                         rhs=uf_s[s][:, j, :], start=True,
                                         stop=True)
                    g2_s[s] = g2
                for s in wave:
                    xf = nxp.tile([P, SETB, P], F32, tag=f"xf{s}",
                                  name=f"xf{s}_{i}")
                    if s % 2 == 0:
                        nc.scalar.activation(xf[:], g2_s[s][:], ACT_COPY,
                                             scale=1.0)
                    else:
                        cp(xf[:], g2_s[s][:])
                    xf_s[s] = xf

        # Q = B @ X with B^T = (2+e)S + (eI12 - P) = 0.5(2+e) D + c32
        q_s = []
        for w0 in range(0, NSETS, 3):
            wave = range(w0, w0 + 3)
            bt_s, gq_s = {}, {}
            for s in wave:
                bt = scr.tile([P, SETB, P], F32, tag=f"sc{s % 3}")
                stt(bt[:], d_s[s][:], float(0.5 * (2.0 + EPS)), c32_s[s][:],
                    MUL, ADD)
                bt_s[s] = bt
            for s in wave:
                g = ps_f.tile([P, SETB, P], F32, tag="gf")
                for j in range(SETB):
                    nc.tensor.matmul(g[:, j, :], lhsT=bt_s[s][:, j, :],
                                     rhs=xf_s[s][:, j, :], start=True,
                                     stop=True)
                gq_s[s] = g
            for s in wave:
                qset = qpool.tile([P, SETB, P], MAIN_DT, tag=f"q{s}",
                                  name=f"q{s}")
                if s % 2 == 0:
                    nc.scalar.activation(qset[:], gq_s[s][:], ACT_COPY,
                                         scale=1.0)
                else:
                    cp(qset[:], gq_s[s][:])
                q_s.append(qset)

        def q_lhsT(n):
            return q_s[n // SETB][:, n % SETB, :]

        # ---------- Phase B: FT[og] <- Q^T @ FT[og] (in place) ----------
        for og in range(O_TILES):
            part = og // 2
            for k in range(NBLK):
                fg = ps_f.tile([P, 512], F32, tag="gf")
                nc.tensor.matmul(fg[:], lhsT=q_lhsT(part * NBLK + k),
                                 rhs=FT[og][:, k, :], start=True, stop=True)
                if k % 2 == 0:
                    nc.scalar.activation(FT[og][:, k, :], fg[:], ACT_COPY,
                                         scale=1.0)
                else:
                    cp(FT[og][:, k, :], fg[:])

        _CACHE["xt_pre"] = xt_pre

    # ---------- Phase C: matmul stream at the 16-bit PE roofline ----------
    xt_pre = _CACHE.pop("xt_pre")
    with tc.tile_pool(name="ps_out", bufs=6, space="PSUM") as ps_out:
        for mt in range(M_TILES):
            la = mt + LOOKAHEAD
            if PREFETCH <= la < M_TILES:
                xt_pre.append(emit_xprep(la))
            xt = xt_pre[mt]
            for og in range(O_TILES):
                po = ps_out.tile([P, 512], F32, tag="po",
                                 name=f"po{mt}_{og}")
                for k in range(NBLK):
                    nc.tensor.matmul(po[:], lhsT=xt[:, k, :],
                                     rhs=FT[og][:, k, :],
                                     start=(k == 0), stop=(k == NBLK - 1))
                ob = obp.tile([P, 512], F32, tag="ob", name=f"ob{mt}_{og}")
                add(ob[:], po[:], bias_bc[:, ts(og, 512)])
                eng = nc.sync if og % 2 == 0 else nc.scalar
                eng.dma_start(out[ts(mt, P), ts(og, 512)], ob[:])


_CACHE = {}


def build():
    if "nc" in _CACHE:
        return _CACHE["nc"]
    import contextlib

    nc = bacc.Bacc("TRN2", target_bir_lowering=False, debug=False)
    with tile.TileContext(nc) as tc:
        with contextlib.ExitStack() as ctx:
            build_body(ctx, tc)
    nc.compile()
    _CACHE["nc"] = nc
    return nc


def make_in_maps(attn_weight, bias, x, q_R, k_R, v_R):
    rmat = np.ascontiguousarray(
        np.concatenate([q_R, k_R, v_R], axis=0), dtype=np.float32)
    w = np.ascontiguousarray(attn_weight, dtype=np.float32)
    b = np.ascontiguousarray(bias, dtype=np.float32)
    return [
        {"x": np.ascontiguousarray(x[c], dtype=np.float32),
         "w": w, "bias": b, "rmat": rmat}
        for c in range(N_CORES)
    ]


def kernel(attn_weight, bias, x, q_R, k_R, v_R, **run_kwargs):
    nc = build()
    in_maps = make_in_maps(attn_weight, bias, x, q_R, k_R, v_R)
    res = run_bass_kernel_spmd(nc, in_maps, core_ids=list(range(N_CORES)),
                               **run_kwargs)
    out = np.stack([res.results[c]["out"] for c in range(N_CORES)], axis=0)
    _CACHE["last_results"] = res
    return out
